# revision 1
# baseline (speedup 1.0000x reference)
"""Trainium2 Bass kernel for nn_EquivariantProductBlock (MACE symmetric contraction + linear).

Strategy (per core, data-parallel over nodes, 1024 nodes/core):
  Host precompute: fold U*W into per-channel polynomial coefficient tensors, symmetrize
  the degree-3 term over its last two indices, and pack per channel-PAIR block-diagonal
  PE weights Wpair [108 K-rows, 80 M-cols] (K = 45 sym-quadratic monomials + 9 linear
  feats per channel; M = (o:4, c01:2, P:10) where P=0..8 are the t[o,p] partials and
  P=9 is the degree-1 path, paired with a constant-1 feature).
  Device: products (DVE, bf16) -> PE transpose to feature-rows -> flipped matmul
  (features stationary w/ FWL, Wpair moving) -> psum t in node-rows -> stage-3
  multiply + tree-reduce (DVE) -> PE transpose to channel-rows -> linear matmuls ->
  PE transpose back + skip-connection add -> store.
"""
import numpy as np

import concourse.bass as bass
import concourse.bacc as bacc
import concourse.mybir as mybir
import concourse.tile as tile
from concourse import bass_utils, masks

F32 = mybir.dt.float32
BF16 = mybir.dt.float16  # 16-bit compute dtype (fp16: 10-bit mantissa)
AX = mybir.AluOpType

NCORES = 8
N = 8192
B = N // NCORES          # nodes per core
MUL = 128
D = 9
NO = 4                   # combined irrep outputs: o=0 -> 0e, o=1..3 -> 1o
M2 = 45
NP = 10                  # P: 0..8 = p, 9 = deg-1 path (x=1)
NPAIR = MUL // 2         # 64
KF = M2 + D              # 54 features per channel
KFP = 64                 # padded per-channel feature block (FWL needs 128-col weights)
KPP = 2 * KFP            # 128 padded pair rows
KP = 2 * KF              # 108 rows per pair
MP = NO * 2 * NP         # 80 cols per pair
PGRP = 6                 # pairs per psum bank in pass-1 (6*80=480 <= 512)
PQ = 3                   # pairs per transpose-psum tile (must divide PGRP)
NT = 8                   # node tiles of 128 per core
NH = 8                   # rounds
NKT = 1                  # node tiles per round
SQRT_MUL = float(np.sqrt(MUL))

# triangle indexing for sym monomials (q <= r), q-major
_TRI = {}
_m2list = []
for _q in range(D):
    for _r in range(_q, D):
        _TRI[(_q, _r)] = len(_m2list)
        _m2list.append((_q, _r))


def _host_weights(U3_0, U2_0, U1_0, U3_1, U2_1, U1_1,
                  W3_0, W2_0, W1_0, W3_1, W2_1, W1_1):
    A3 = np.zeros((MUL, NO, D, D, D), np.float32)
    C2 = np.zeros((MUL, NO, D, D), np.float32)
    C1 = np.zeros((MUL, NO, D), np.float32)
    for (osl, U3, U2, U1, W3, W2, W1) in (
        (slice(0, 1), U3_0, U2_0, U1_0, W3_0, W2_0, W1_0),
        (slice(1, 4), U3_1, U2_1, U1_1, W3_1, W2_1, W1_1),
    ):
        A3[:, osl] = np.einsum("opqrk,kc->copqr", U3, W3)
        C2[:, osl] = np.einsum("opqk,kc->copq", U2, W2)
        C1[:, osl] = np.einsum("opk,kc->cop", U1, W1)
    Atil = np.zeros((MUL, NO, D, M2), np.float32)
    for (q, r), m in _TRI.items():
        Atil[:, :, :, m] = A3[:, :, :, q, r] + (A3[:, :, :, r, q] if q != r else 0.0)
    # per-channel [54, NO, NP]
    Wch = np.zeros((MUL, KF, NO, NP), np.float32)
    Wch[:, 0:M2, :, 0:9] = np.moveaxis(Atil, 3, 1)
    Wch[:, M2:KF, :, 0:9] = np.moveaxis(C2, 3, 1)
    Wch[:, M2:KF, :, 9] = np.moveaxis(C1, 2, 1)
    # pair lhs: rows [c0 54, c1 54]; col = o*20 + c01*10 + P
    Wpair = np.zeros((NPAIR, KPP, MP), np.float32)
    for j in range(NPAIR):
        for c01 in range(2):
            w = Wch[2 * j + c01]  # [54, NO, NP]
            for o in range(NO):
                Wpair[j, KFP * c01:KFP * c01 + KF, o * 20 + c01 * 10:o * 20 + c01 * 10 + NP] = w[:, o, :]
    return Wpair.transpose(1, 0, 2).reshape(KPP, NPAIR * MP)


_CACHE = {}


def _tri_off(p):
    return p * D - p * (p - 1) // 2


def _build(stage=9):
    key = ("nc", stage)
    if key in _CACHE:
        return _CACHE[key]
    nc = bacc.Bacc("TRN2", target_bir_lowering=False, debug=False)
    nf_d = nc.dram_tensor("nf", [B, MUL * D], F32, kind="ExternalInput")
    sc_d = nc.dram_tensor("sc", [B, NO * MUL], F32, kind="ExternalInput")
    wp_d = nc.dram_tensor("wp", [KPP, NPAIR * MP], BF16, kind="ExternalInput")
    lw_d = nc.dram_tensor("lw", [MUL, 2 * MUL], F32, kind="ExternalInput")  # [u, (lw0 v | lw1 v)]
    out_d = nc.dram_tensor("out", [B, NO * MUL], F32, kind="ExternalOutput")

    with tile.TileContext(nc) as tc:
        with (
            tc.tile_pool(name="const", bufs=1) as constp,
            tc.tile_pool(name="xf", bufs=3) as xfp,
            tc.tile_pool(name="xbig", bufs=4) as xbigp,
            tc.tile_pool(name="feat", bufs=PGRP + 1) as featp,
            tc.tile_pool(name="ts", bufs=4) as tsp,  # 4 alive per half
            tc.tile_pool(name="st3", bufs=2) as st3p,
            tc.tile_pool(name="fs", bufs=2) as fsp,
            tc.tile_pool(name="ys", bufs=2) as ysp,
            tc.tile_pool(name="yout", bufs=2) as youtp,
            tc.tile_pool(name="scp", bufs=2) as scp,
            tc.tile_pool(name="psT", bufs=3, space=bass.MemorySpace.PSUM) as psTp,
            tc.tile_pool(name="psG", bufs=2, space=bass.MemorySpace.PSUM) as psGp,
            tc.tile_pool(name="psO", bufs=2, space=bass.MemorySpace.PSUM) as psOp,
            tc.tile_pool(name="psY", bufs=1, space=bass.MemorySpace.PSUM) as psYp,
            tc.tile_pool(name="psYT", bufs=1, space=bass.MemorySpace.PSUM) as psYTp,
        ):
            # constants
            ident_b = constp.tile([128, 128], BF16, tag="idb")
            ident_f = constp.tile([128, 128], F32, tag="idf")
            masks.make_identity(nc, ident_b[:])
            masks.make_identity(nc, ident_f[:])
            wpS = constp.tile([KPP, NPAIR * MP], BF16, tag="wp")
            nc.sync.dma_start(wpS[:], wp_d.ap())
            lwS = constp.tile([MUL, 2 * MUL], F32, tag="lw")
            nc.sync.dma_start(lwS[:], lw_d.ap())

            for h in range(NH):
                xbig = []
                tSs = []
                for k in range(NKT):
                    g = h * NKT + k
                    rows = slice(g * 128, (g + 1) * 128)
                    xf = xfp.tile([128, MUL * D], F32, tag="xf")
                    if g >= 2:
                        # bridge: collapse the load-DMA's (readers + WAW) waits
                        # into a single DVE-semaphore wait (DMA 1-wait limit)
                        nc.scalar.copy(xf[0:1, 0:1], ident_f[0:1, 0:1])
                    nc.sync.dma_start(xf[:], nf_d.ap()[rows, :])
                    # xbig layout: [b, pair, c01, f] -- pair's features contiguous
                    xb = xbigp.tile([128, NPAIR, 2, KFP], BF16, tag="xbig")
                    nc.gpsimd.memset(xb[:, :, :, KF:KF + 1], 1.0)   # P=9 constant-1 path
                    nc.gpsimd.memset(xb[:, :, :, KF + 1:KFP], 0.0)
                    xL = xb[:, :, :, M2:KF]  # [128, 64, 2, 9] view
                    nc.scalar.copy(
                        xL[:, :, :, 0].rearrange("b p c -> b (p c)"), xf[:, 0:MUL]
                    )
                    nc.scalar.copy(
                        xL[:, :, :, 1:4].rearrange("b p c j -> b (p c) j"),
                        xf[:, MUL:4 * MUL].rearrange("b (c j) -> b c j", j=3),
                    )
                    nc.scalar.copy(
                        xL[:, :, :, 4:9].rearrange("b p c j -> b (p c) j"),
                        xf[:, 4 * MUL:].rearrange("b (c j) -> b c j", j=5),
                    )
                    # sym quadratic monomials into cols 0..44 of each channel block
                    for p in range(D):
                        npair_p = D - p
                        eng = nc.gpsimd if p < 2 else nc.vector
                        eng.tensor_tensor(
                            xb[:, :, :, _tri_off(p):_tri_off(p) + npair_p],
                            xL[:, :, :, p:p + 1].broadcast_to([128, NPAIR, 2, npair_p]),
                            xL[:, :, :, p:D],
                            AX.mult,
                        )
                    xbig.append(xb)
                    tSs.append(tsp.tile([128, NPAIR * MP], BF16, tag="ts", name=f"ts{k}"))

                # pass-1 per pair group: transpose features, matmul with Wpair
                W512 = NKT * 128
                ngrp = (NPAIR + PGRP - 1) // PGRP
                for gg in range(ngrp):
                    pj0 = gg * PGRP
                    pj1 = min(pj0 + PGRP, NPAIR)
                    feats = {}
                    for j0 in range(pj0, pj1, PQ):
                        j1 = min(j0 + PQ, pj1)
                        psT = psTp.tile([KPP, PQ * NKT * 128], BF16, tag="psT")
                        for dj in range(j1 - j0):
                            for k in range(NKT):
                                src = xbig[k][:, j0 + dj, :, :].rearrange("b c f -> b (c f)")
                                nc.tensor.transpose(
                                    psT[:, (dj * NKT + k) * 128:(dj * NKT + k + 1) * 128],
                                    src, ident_b[:],
                                )
                        fS = featp.tile([KPP, PQ * NKT * 128], BF16, tag="feat")
                        ceng = nc.vector if (j0 // PQ) % 2 == 0 else nc.scalar
                        if ceng is nc.vector:
                            nc.vector.tensor_copy(fS[:, 0:(j1 - j0) * NKT * 128], psT[:, 0:(j1 - j0) * NKT * 128])
                        else:
                            nc.scalar.copy(fS[:, 0:(j1 - j0) * NKT * 128], psT[:, 0:(j1 - j0) * NKT * 128])
                        feats[j0] = fS
                    for k in range(NKT):
                        psG = psGp.tile([128, (pj1 - pj0) * MP], F32, tag="psG")
                        for ji, j in enumerate(range(pj0, pj1)):
                            fS = feats[pj0 + (ji // PQ) * PQ]
                            nc.tensor.matmul(
                                psG[:, ji * MP:(ji + 1) * MP],
                                fS[:, ((ji % PQ) * NKT + k) * 128:((ji % PQ) * NKT + k + 1) * 128],
                                wpS[:, j * MP:(j + 1) * MP],
                                start=True,
                                stop=True,
                            )
                        nc.scalar.copy(tSs[k][:, pj0 * MP:pj1 * MP], psG[:])

                # stage-3: multiply by x (P=9 pairs with 1.0) and tree-reduce over P
                fSc = fsp.tile([MUL, NO * W512], F32, tag="fs")  # [c, (o, roundb)]
                for k in range(NKT):
                    tS = tSs[k]
                    prod = st3p.tile([128, NPAIR * MP], BF16, tag="prod")
                    in1 = xbig[k][:, :, :, M2:M2 + NP]
                    in0 = tS[:].rearrange("b (pr oc P) -> b pr oc P", pr=NPAIR, P=NP)
                    prv = prod[:].rearrange("b (pr oc P) -> b pr oc P", pr=NPAIR, P=NP)
                    for o in range(NO):
                        nc.vector.tensor_tensor(
                            prv[:, :, o * 2:o * 2 + 2, :],
                            in0[:, :, o * 2:o * 2 + 2, :],
                            in1, AX.mult,
                        )
                    pr5 = prod[:].rearrange("b (g P) -> b g P", P=NP)
                    t1 = st3p.tile([128, 512 * 4], BF16, tag="s1")
                    t1v = t1[:].rearrange("b (g P) -> b g P", P=4)
                    nc.vector.tensor_tensor(t1v, pr5[:, :, 0:4], pr5[:, :, 4:8], AX.add)
                    t2 = st3p.tile([128, 512 * 2], BF16, tag="s2")
                    t2v = t2[:].rearrange("b (g P) -> b g P", P=2)
                    nc.vector.tensor_tensor(t2v, t1v[:, :, 0:2], t1v[:, :, 2:4], AX.add)
                    t3 = st3p.tile([128, 512 * 2], BF16, tag="s3")
                    t3v = t3[:].rearrange("b (g P) -> b g P", P=2)
                    nc.vector.tensor_tensor(t3v, t2v, pr5[:, :, 8:10], AX.add)
                    out3 = st3p.tile([128, NO * MUL], F32, tag="out3")
                    o3v = out3[:].rearrange("b (o pr c) -> b pr o c", o=NO, pr=NPAIR, c=2)
                    ta = t3v[:, :, 0].rearrange("b (pr o c) -> b pr o c", pr=NPAIR, o=NO, c=2)
                    tb = t3v[:, :, 1].rearrange("b (pr o c) -> b pr o c", pr=NPAIR, o=NO, c=2)
                    nc.vector.tensor_tensor(o3v, ta, tb, AX.add)
                    # transpose out3 -> channel rows
                    for o in range(NO):
                        psO = psOp.tile([MUL, 128], F32, tag="psO")
                        nc.tensor.transpose(
                            psO[:], out3[:, o * MUL:(o + 1) * MUL], ident_f[:]
                        )
                        nc.scalar.copy(
                            fSc[:, o * W512 + k * 128:o * W512 + (k + 1) * 128], psO[:]
                        )

                # per-irrep channel-mixing linear
                yS = ysp.tile([MUL, NO * W512], F32, tag="ys")  # [v, (o, roundb)]
                for o in range(NO):
                    psY = psYp.tile([MUL, W512], F32, tag="psY")
                    lhs = lwS[:, 0:MUL] if o == 0 else lwS[:, MUL:2 * MUL]
                    nc.tensor.matmul(
                        psY[:], lhs, fSc[:, o * W512:(o + 1) * W512],
                        start=True, stop=True,
                    )
                    nc.scalar.copy(yS[:, o * W512:(o + 1) * W512], psY[:])

                # back to node rows, scale + skip connection, store
                for k in range(NKT):
                    g = h * NKT + k
                    rows = slice(g * 128, (g + 1) * 128)
                    scT = scp.tile([128, NO * MUL], F32, tag="sc")
                    if g >= 2:
                        nc.vector.tensor_copy(scT[0:1, 0:1], ident_f[0:1, 0:1])
                    nc.gpsimd.dma_start(scT[:], sc_d.ap()[rows, :])
                    yo = youtp.tile([128, NO * MUL], F32, tag="yout")
                    for o in range(NO):
                        psYT = psOp.tile([128, MUL], F32, tag="psO")
                        nc.tensor.transpose(
                            psYT[:], yS[:, o * W512 + k * 128:o * W512 + (k + 1) * 128],
                            ident_f[:],
                        )
                        if o == 0:
                            dst = yo[:, 0:MUL]
                            scs = scT[:, 0:MUL]
                        else:
                            dst = yo[:, MUL:].rearrange("b (v j) -> b j v", j=3)[:, o - 1, :]
                            scs = scT[:, MUL:].rearrange("b (v j) -> b j v", j=3)[:, o - 1, :]
                        nc.vector.scalar_tensor_tensor(
                            dst, psYT[:], 1.0 / SQRT_MUL, scs, AX.mult, AX.add
                        )
                    nc.sync.dma_start(out_d.ap()[rows, :], yo[:])

    nc.compile()
    _CACHE[key] = nc
    return nc


def kernel(node_feats, sc, U3_0, U2_0, U1_0, U3_1, U2_1, U1_1,
           W3_0, W2_0, W1_0, W3_1, W2_1, W1_1, lin_w0, lin_w1):
    args = {k: np.asarray(v, np.float32) for k, v in dict(
        U3_0=U3_0, U2_0=U2_0, U1_0=U1_0, U3_1=U3_1, U2_1=U2_1, U1_1=U1_1,
        W3_0=W3_0, W2_0=W2_0, W1_0=W1_0, W3_1=W3_1, W2_1=W2_1, W1_1=W1_1,
    ).items()}
    wp = _host_weights(**args).astype(np.float32)
    wp_bf = wp.astype(np.float16)
    lw = np.concatenate(
        [np.asarray(lin_w0, np.float32), np.asarray(lin_w1, np.float32)], axis=1
    )
    nf = np.ascontiguousarray(np.asarray(node_feats, np.float32))
    scf = np.ascontiguousarray(np.asarray(sc, np.float32))

    nc = _build()
    in_maps = []
    for c in range(NCORES):
        in_maps.append({
            "nf": nf[c * B:(c + 1) * B],
            "sc": scf[c * B:(c + 1) * B],
            "wp": wp_bf,
            "lw": lw,
        })
    global _last_in_maps
    _last_in_maps = in_maps
    res = bass_utils.run_bass_kernel_spmd(nc, in_maps, core_ids=list(range(NCORES)))
    out = np.concatenate([r["out"] for r in res.results], axis=0)
    return out.astype(np.float32)


_last_in_maps = None



# revision 3
# speedup vs baseline: 1.3866x; 1.3866x over previous
"""Trainium2 Bass kernel v2 for nn_EquivariantProductBlock (MACE symmetric contraction + linear).

Strategy (per core, data-parallel over nodes, 1024 nodes/core, 8 tiles of 128):
  Host: fully symmetrize the cubic term over (p,q,r); head/tail split with
  head = vars 0..5, tail = 6..8. Per-channel features (K=64 exactly):
    f 0..44  : 45 symmetric quadratic monomials m_qr (q<=r, q-major)
    f 45..54 : 10 pure-tail cubic monomials x_a*m_bc (6<=a<=b<=c)
    f 55..63 : 9 linear features x_p
  Per-pair stationary weights Wpair [128 rows (f,c01), 56 cols (o,t,c01)]:
    t=0..5 are head partials (stage-3 multiplies by x_t and reduces),
    t=6 is the direct path (no multiply). lin weights carry 1/sqrt(128).
  Device per tile: Act lays out x (bf16, c01-last) -> DVE builds quads+cubics
  at 2x (c01-packed last dim) -> PE transposes 8-pair blocks to K-rows (psT,
  1 PSUM bank) -> fS copies (DMA/Pool/DVE mix) -> flipped matmuls (features
  stationary, Wpair moving, 56 cols) -> psG f32 -> Act copies to tG bf16 ->
  DVE stage-3 (1 prod op + 4 reduce ops, all 2x) -> out3 bf16 -> PE transpose
  to channel rows -> fSc -> flipped linear (features stationary, lw moving:
  output lands in node rows, no back-transpose) -> Pool adds skip -> DMA out.
"""
import itertools
import numpy as np

import concourse.bass as bass
import concourse.bacc as bacc
import concourse.mybir as mybir
import concourse.tile as tile
from concourse import bass_utils, masks

F32 = mybir.dt.float32
BF16 = mybir.dt.float16
AX = mybir.AluOpType

NCORES = 8
N = 8192
B = N // NCORES          # nodes per core
MUL = 128
D = 9
NO = 4                   # combined irrep outputs: o=0 -> 0e, o=1..3 -> 1o
HEAD = 6                 # head vars 0..5; tail 6..8
NT = HEAD + 1            # 6 partials + 1 direct
KF = 64                  # feats per channel: 45 quad + 10 cubic + 9 linear
NPAIR = MUL // 2         # 64
MP = NO * NT * 2         # 56 cols per pair: (o, t, c01)
PGRP = 8                 # pairs per psG bank (8*56*4B = 1792 <= 2048)
TGRP = 8                 # pairs per psT bank (8*128*2B = 2048)
NTILE = 8                # node tiles of 128 per core
SQRT_MUL = float(np.sqrt(MUL))

_TRI = {}
for _q in range(D):
    for _r in range(_q, D):
        _TRI[(_q, _r)] = len(_TRI)
CUBES = [(a, b, c) for a in range(HEAD, D) for b in range(a, D) for c in range(b, D)]
CUBE_IDX = {t: i for i, t in enumerate(CUBES)}


def _host_weights(U3_0, U2_0, U1_0, U3_1, U2_1, U1_1,
                  W3_0, W2_0, W1_0, W3_1, W2_1, W1_1):
    A3 = np.zeros((MUL, NO, D, D, D), np.float32)
    C2 = np.zeros((MUL, NO, D, D), np.float32)
    C1 = np.zeros((MUL, NO, D), np.float32)
    for (osl, U3, U2, U1, W3, W2, W1) in (
        (slice(0, 1), U3_0, U2_0, U1_0, W3_0, W2_0, W1_0),
        (slice(1, 4), U3_1, U2_1, U1_1, W3_1, W2_1, W1_1),
    ):
        A3[:, osl] = np.einsum("opqrk,kc->copqr", U3, W3)
        C2[:, osl] = np.einsum("opqk,kc->copq", U2, W2)
        C1[:, osl] = np.einsum("opk,kc->cop", U1, W1)
    W = np.zeros((MUL, KF, NO, NT), np.float32)
    for a in range(D):
        for b in range(a, D):
            for c in range(b, D):
                S = 0.0
                for perm in set(itertools.permutations((a, b, c))):
                    S = S + A3[:, :, perm[0], perm[1], perm[2]]
                if a < HEAD:
                    W[:, _TRI[(b, c)], :, a] += S
                else:
                    W[:, 45 + CUBE_IDX[(a, b, c)], :, HEAD] += S
    for a in range(D):
        for b in range(a, D):
            S = C2[:, :, a, b] + (C2[:, :, b, a] if a != b else 0.0)
            if a < HEAD:
                W[:, 55 + b, :, a] += S
            else:
                W[:, _TRI[(a, b)], :, HEAD] += S
    for p in range(D):
        W[:, 55 + p, :, HEAD] += C1[:, :, p]
    # Wpair [128 rows (f,c01), NPAIR*MP], col = j*MP + o*NT*2 + t*2 + c01
    Wpair = np.zeros((2 * KF, NPAIR * MP), np.float32)
    for j in range(NPAIR):
        for c01 in range(2):
            w = W[2 * j + c01]  # [KF, NO, NT]
            for f in range(KF):
                Wpair[2 * f + c01, j * MP + c01:(j + 1) * MP:2] = w[f].reshape(-1)
    return Wpair


_CACHE = {}

# engine assignment for the 8 feature-group stagings per tile (tunable):
# 'x' = DMA xbar transpose (no PE transpose, no copy)
# 'p'/'a'/'v' = PE transpose + PSUM->SBUF copy on Pool/Act/DVE
FS_ENG = "xxxxxaaa"
# engine for the 8 psG->tG copies per tile: 'a' = Act, 'p' = Pool, 'v' = DVE
PSG_ENG = "aaaaaaaa"
LAG = 2                  # software-pipeline depth (rounds) for the back stage
PSG_BUFS = 4
OLAG = 4                 # extra-delayed out-DMA emission (avoids SP queue jam)


def _tri_off(p):
    return p * D - p * (p - 1) // 2


def _build(stage=9):
    key = ("nc", stage)
    if key in _CACHE:
        return _CACHE[key]
    nc = bacc.Bacc("TRN2", target_bir_lowering=False, debug=False)
    nf_d = nc.dram_tensor("nf", [B, MUL * D], F32, kind="ExternalInput")
    sc_d = nc.dram_tensor("sc", [B, NO * MUL], F32, kind="ExternalInput")
    wp_d = nc.dram_tensor("wp", [2 * KF, NPAIR * MP], BF16, kind="ExternalInput")
    lw_d = nc.dram_tensor("lw", [MUL, 2 * MUL], BF16, kind="ExternalInput")
    out_d = nc.dram_tensor("out", [B, NO * MUL], F32, kind="ExternalOutput")

    NG = NPAIR // PGRP  # 8 psG groups per tile

    with tile.TileContext(nc) as tc:
        with (
            tc.tile_pool(name="const", bufs=1) as constp,
            tc.tile_pool(name="xf", bufs=3) as xfp,
            tc.tile_pool(name="xbig", bufs=LAG + 3) as xbigp,
            tc.tile_pool(name="fs", bufs=10) as fsp,
            tc.tile_pool(name="tg", bufs=LAG + 1) as tgp,
            tc.tile_pool(name="st3", bufs=2) as st3p,
            tc.tile_pool(name="fsc", bufs=2) as fscp,
            tc.tile_pool(name="scp", bufs=LAG + 2) as scp,
            tc.tile_pool(name="yout", bufs=OLAG) as youtp,
            tc.tile_pool(name="psT", bufs=2, space=bass.MemorySpace.PSUM) as psTp,
            tc.tile_pool(name="psG", bufs=PSG_BUFS, space=bass.MemorySpace.PSUM) as psGp,
            tc.tile_pool(name="psO", bufs=1, space=bass.MemorySpace.PSUM) as psOp,
            tc.tile_pool(name="psF", bufs=1, space=bass.MemorySpace.PSUM) as psFp,
        ):
            ident_b = constp.tile([128, 128], BF16, tag="idb")
            masks.make_identity(nc, ident_b[:])
            wpS = constp.tile([2 * KF, NPAIR * MP], BF16, tag="wp")
            nc.sync.dma_start(wpS[:], wp_d.ap())
            lwS = constp.tile([MUL, 2 * MUL], BF16, tag="lw")
            nc.sync.dma_start(lwS[:], lw_d.ap())

            xfs = {}
            scs = {}
            state = {}
            youts = {}

            def load_xf(g, bridge):
                rows = slice(g * 128, (g + 1) * 128)
                xf = xfp.tile([128, MUL * D], F32, tag="xf")
                if bridge:
                    nc.vector.tensor_copy(xf[0:1, 0:1], ident_b[0:1, 0:1])
                nc.sync.dma_start(xf[:], nf_d.ap()[rows, :])
                xfs[g] = xf

            def load_sc(g, bridge):
                rows = slice(g * 128, (g + 1) * 128)
                scT = scp.tile([128, NO * MUL], F32, tag="sc")
                if bridge:
                    nc.gpsimd.tensor_copy(scT[0:1, 0:1], ident_b[0:1, 0:1])
                nc.sync.dma_start(scT[:], sc_d.ap()[rows, :])
                scs[g] = scT

            def front(g):
                xf = xfs.pop(g)
                # xbig layout: [b, pair, f(64), c01(2)] (c01-last for DVE 2x)
                xb = xbigp.tile([128, NPAIR, KF, 2], BF16, tag="xbig")
                xL = xb[:, :, 55:64, :]  # linear feats [b, pr, j, c01]
                # xf cols: 0e c | 1o (c,j) | 2e (c,j); c = 2*pr + c01
                nc.gpsimd.tensor_copy(
                    xL[:, :, 0, :], xf[:, 0:MUL].rearrange("b (p c) -> b p c", c=2)
                )
                nc.gpsimd.tensor_copy(
                    xL[:, :, 1:4, :],
                    xf[:, MUL:4 * MUL].rearrange("b (p c j) -> b p j c", p=NPAIR, c=2),
                )
                nc.gpsimd.tensor_copy(
                    xL[:, :, 4:9, :],
                    xf[:, 4 * MUL:].rearrange("b (p c j) -> b p j c", p=NPAIR, c=2),
                )
                # quads: f 0..44, q-major triangle
                for p in range(D):
                    npair_p = D - p
                    nc.vector.tensor_tensor(
                        xb[:, :, _tri_off(p):_tri_off(p) + npair_p, :],
                        xL[:, :, p:p + 1, :].broadcast_to([128, NPAIR, npair_p, 2]),
                        xL[:, :, p:D, :],
                        AX.mult,
                    )
                # tail cubics: f 45..54 = x_a * m_bc, a,b,c >= 6
                # quad rows tri(6,6)..tri(8,8) = f 39..44 (contiguous)
                for i, (p, n) in enumerate(((6, 6), (7, 3), (8, 1))):
                    off = _tri_off(p)
                    dst = 45 + (0, 6, 9)[i]
                    nc.vector.tensor_tensor(
                        xb[:, :, dst:dst + n, :],
                        xL[:, :, p:p + 1, :].broadcast_to([128, NPAIR, n, 2]),
                        xb[:, :, off:off + n, :],
                        AX.mult,
                    )

                # features -> K-rows: 8 staging groups of 8 pairs; psG spans 2
                # groups (16 pairs) as a 2-bank tile with 64B pad per bank
                tG = tgp.tile([128, NPAIR, NO, NT, 2], BF16, tag="tg")
                for gg in range(NG):
                    j0 = gg * TGRP
                    eng = FS_ENG[gg]
                    fS = fsp.tile([128, TGRP, 128], BF16, tag="fs")
                    if eng == "x":
                        nc.sync.dma_start_transpose(
                            fS[:],
                            xb[:, j0:j0 + TGRP, :, :].rearrange("b p f c -> b (p f c)"),
                        )
                    else:
                        psT = psTp.tile([128, TGRP * 128], BF16, tag="psT")
                        for dj in range(TGRP):
                            nc.tensor.transpose(
                                psT[:, dj * 128:(dj + 1) * 128],
                                xb[:, j0 + dj, :, :].rearrange("b f c -> b (f c)"),
                                ident_b[:],
                            )
                        fv = fS[:].rearrange("b p n -> b (p n)")
                        if eng == "p":
                            nc.gpsimd.tensor_copy(fv, psT[:])
                        elif eng == "a":
                            nc.scalar.copy(fv, psT[:])
                        else:
                            nc.vector.tensor_copy(fv, psT[:])
                    # pair matmuls: features stationary, Wpair moving
                    psG = psGp.tile([128, PGRP * MP], F32, tag="psG")
                    for dj in range(TGRP):
                        j = j0 + dj
                        nc.tensor.matmul(
                            psG[:, dj * MP:(dj + 1) * MP],
                            fS[:, dj, :],
                            wpS[:, j * MP:(j + 1) * MP],
                            start=True, stop=True,
                        )
                    dstv = tG[:, j0:j0 + TGRP, :, :, :].rearrange(
                        "b p o t c -> b (p o t c)")
                    peng = PSG_ENG[gg]
                    if peng == "a":
                        nc.scalar.copy(dstv, psG[:])
                    elif peng == "p":
                        nc.gpsimd.tensor_copy(dstv, psG[:])
                    else:
                        nc.vector.tensor_copy(dstv, psG[:])
                state[g] = (xb, tG)

            def back(g):
                xb, tG = state.pop(g)
                # stage-3 (DVE, all 2x): prod + tree reduce
                prod = st3p.tile([128, NPAIR, NO, HEAD, 2], BF16, tag="prod")
                nc.vector.tensor_tensor(
                    prod[:],
                    tG[:, :, :, 0:HEAD, :],
                    xb[:, :, None, 55:55 + HEAD, :]
                        .broadcast_to([128, NPAIR, NO, HEAD, 2]),
                    AX.mult,
                )
                pv = prod[:].rearrange("b p o (u v) c -> b p o u v c", u=3, v=2)
                s3 = st3p.tile([128, NPAIR, NO, 3, 2], BF16, tag="s3")
                nc.vector.tensor_tensor(s3[:], pv[:, :, :, :, 0, :], pv[:, :, :, :, 1, :], AX.add)
                d1 = st3p.tile([128, NPAIR, NO, 2], BF16, tag="d1")
                nc.vector.tensor_tensor(d1[:], s3[:, :, :, 2, :], tG[:, :, :, HEAD, :], AX.add)
                # out3 [b, (o, pr, c01)] = channel-major per o for transpose
                out3 = st3p.tile([128, NO, NPAIR, 2], BF16, tag="out3")
                o3v = out3[:].rearrange("b o p c -> b p o c")
                nc.vector.tensor_tensor(o3v, s3[:, :, :, 0, :], s3[:, :, :, 1, :], AX.add)
                nc.gpsimd.tensor_tensor(o3v, o3v, d1[:], AX.add)

                # linear: transpose to channel rows, then flipped matmul
                psO = psOp.tile([128, NO * 128], BF16, tag="psO")
                for o in range(NO):
                    nc.tensor.transpose(
                        psO[:, o * 128:(o + 1) * 128],
                        out3[:, o, :, :].rearrange("b p c -> b (p c)"),
                        ident_b[:],
                    )
                fSc = fscp.tile([128, NO * 128], BF16, tag="fsc")
                nc.scalar.copy(fSc[:], psO[:])
                psF = psFp.tile([128, NO * 128], F32, tag="psF")
                for o in range(NO):
                    lhs = lwS[:, 0:MUL] if o == 0 else lwS[:, MUL:2 * MUL]
                    nc.tensor.matmul(
                        psF[:, o * 128:(o + 1) * 128],
                        fSc[:, o * 128:(o + 1) * 128],
                        lhs,
                        start=True, stop=True,
                    )

                # skip connection + store (Pool)
                rows = slice(g * 128, (g + 1) * 128)
                scT = scs.pop(g)
                yo = youtp.tile([128, NO * MUL], F32, tag="yout")
                nc.vector.tensor_tensor(
                    yo[:, 0:MUL], psF[:, 0:MUL], scT[:, 0:MUL], AX.add
                )
                nc.vector.tensor_tensor(
                    yo[:, MUL:].rearrange("b (v j) -> b j v", j=3),
                    psF[:, MUL:].rearrange("b (j v) -> b j v", j=3),
                    scT[:, MUL:].rearrange("b (v j) -> b j v", j=3),
                    AX.add,
                )
                youts[g] = yo

            def emit_out(g):
                rows = slice(g * 128, (g + 1) * 128)
                # emitted OLAG rounds late: yo(g) is long done, so this DMA
                # never blocks the SP queue with a wait
                nc.sync.dma_start(out_d.ap()[rows, :], youts.pop(g)[:])

            # software-pipelined schedule: front(g), back(g-LAG) per round,
            # with xf prefetched 2 rounds and sc 1 round ahead
            load_xf(0, False)
            load_xf(1, False)
            load_sc(0, False)
            for r in range(NTILE + OLAG):
                if r < NTILE:
                    if r + 2 < NTILE:
                        load_xf(r + 2, True)
                    if r + 1 < NTILE:
                        load_sc(r + 1, r >= 1)
                    front(r)
                if LAG <= r < NTILE + LAG:
                    back(r - LAG)
                if r >= OLAG:
                    emit_out(r - OLAG)

    nc.compile()
    _CACHE[key] = nc
    return nc


def kernel(node_feats, sc, U3_0, U2_0, U1_0, U3_1, U2_1, U1_1,
           W3_0, W2_0, W1_0, W3_1, W2_1, W1_1, lin_w0, lin_w1):
    args = {k: np.asarray(v, np.float32) for k, v in dict(
        U3_0=U3_0, U2_0=U2_0, U1_0=U1_0, U3_1=U3_1, U2_1=U2_1, U1_1=U1_1,
        W3_0=W3_0, W2_0=W2_0, W1_0=W1_0, W3_1=W3_1, W2_1=W2_1, W1_1=W1_1,
    ).items()}
    wp_bf = _host_weights(**args).astype(np.float16)
    lw = np.concatenate(
        [np.asarray(lin_w0, np.float32), np.asarray(lin_w1, np.float32)], axis=1
    ) / SQRT_MUL
    lw_bf = lw.astype(np.float16)
    nf = np.ascontiguousarray(np.asarray(node_feats, np.float32))
    scf = np.ascontiguousarray(np.asarray(sc, np.float32))

    nc = _build()
    in_maps = []
    for c in range(NCORES):
        in_maps.append({
            "nf": nf[c * B:(c + 1) * B],
            "sc": scf[c * B:(c + 1) * B],
            "wp": wp_bf,
            "lw": lw_bf,
        })
    global _last_in_maps
    _last_in_maps = in_maps
    res = bass_utils.run_bass_kernel_spmd(nc, in_maps, core_ids=list(range(NCORES)))
    out = np.concatenate([r["out"] for r in res.results], axis=0)
    return out.astype(np.float32)


_last_in_maps = None
